# revision 16
# baseline (speedup 1.0000x reference)
"""Trainium2 Bass kernel for nn_Encoder_Flows (3-layer dense GCN message passing).

Math per graph (reference):
    A = flows [N, N];  deg[c] = sum_r A[r, c];  dinv = rsqrt(deg)
    L(x, W, b) = dinv * (A^T @ (dinv * (x @ W))) + b
    out = L(L(L(A, W1, b1), W2, b2), W3, b3)          # [N, 128]

Algebra: with M = diag(dinv) A^T diag(dinv), node-dim M commutes with the
feature-dim weights, so (bias-free) out = M^3 (A W1) (W2 W3).  Key trick of
this version: the degree normalization is folded into the shipped matrix on
the host:  Ahat = D A D  =>  M = Ahat^T exactly.  Every M-apply is then a
plain fp8 DoubleRow matmul chain  t_k = Ahat^T w_{k-1}  with NO per-step
dinv scaling on device (the vector engine only does psum->sbuf casts and the
eps residuals).  The U phase u = A W1 runs off Ahat^T strips with
W1g = D^{-1} W1 folded on the host: (D^{-1}W1)^T Ahat^T = u^T D, undone by a
per-node act scale when u is quantized.

fp8 quantization corrections (node-mean noise is amplified ~sqrt(N) by the
adjacency's Perron mode):
  - W1g is dither-quantized per column (error col-sums ~0).
  - w1/w2's quantization residual col-means mu_k are measured on device and
    applied as exact rank-1 psum accumulations  m1 (x) mu_k  into the next
    phase, where m1 = colsum(Ahat_q) is shipped from the host (a K=1 matmul
    appended to the accumulation group -- no vector work).
Scales: Ahat*2^16, W1g*2^6, w*2^7(t units), mu*2^-11, W23*2^2, out=psum*2^-15.

Performance design (measured baseline: PE never left the 1.2GHz mid p-state;
the 2.4GHz p-state needs >3us of gapless tensor-queue execution):
  - data: 2 graphs/core, Ahat shipped in both layouts (natural row-packed +
    transposed strips), 1MB c-chunk DMAs so each phase is chunk-paced;
    output in bf16.  ~17.9MB on a ~400GB/s DMA bus dominates the schedule.
  - the tensor queue is padded with junk DoubleRow "filler" matmuls wherever
    it would otherwise idle (DMA-paced stretches, post-processing trails) so
    the PE holds the full 2.4GHz clock; at full clock a 512-col DR matmul
    retires in ~107ns.
  - posts per chunk: vector casts psum->bf16, PE transposes 128x128 blocks,
    scalar quantizes to fp8 (act, with the per-node scale vector for U),
    vector computes eps = x - w in one scalar_tensor_tensor op.
"""

import sys
from contextlib import ExitStack

import numpy as np

for _p in ("/opt/trn_rl_repo", "/opt/pypackages"):
    if _p not in sys.path:
        sys.path.append(_p)

import ml_dtypes

B, N, P = 16, 2048, 128
NB = N // P          # 16 node 128-blocks
KB = N // (2 * P)    # 8 DoubleRow pair-blocks (256 rows each)
NCORES = 8
GPC = B // NCORES    # graphs per core
D = 128              # feature width carried through the fused chain
CH = 512             # psum chunk (one bank of fp32)
NCH = N // CH

_COMPILED = {}

# filler counts (each ~107ns of junk DR matmul at full clock), tuned to the
# DMA pacing of the schedule
FILL = dict(head=25, u0=4, u0t1=5, t1=4, t1mu=3, t2t3=0,
            u1a=0, u1b=4, u1t1=3, t3tail=3)


def _build():
    import concourse.mybir as mybir
    import concourse.tile as tile
    from concourse import bacc

    f32 = mybir.dt.float32
    bf16 = mybir.dt.bfloat16
    fp8 = mybir.dt.float8e4
    DR = mybir.MatmulPerfMode.DoubleRow
    MUL = mybir.AluOpType.mult
    SUB = mybir.AluOpType.subtract
    COPY = mybir.ActivationFunctionType.Copy

    nc = bacc.Bacc("TRN2", target_bir_lowering=False)
    AH_d = nc.declare_dram_parameter("AH", [GPC, NCH, P, KB, 2, CH], fp8,
                                     isOutput=False)
    AT_d = nc.declare_dram_parameter("AT", [GPC, NCH, P, KB, 2, CH], fp8,
                                     isOutput=False)
    W1G_d = nc.declare_dram_parameter("W1G", [GPC, KB, P, 2, D], fp8,
                                      isOutput=False)
    W23_d = nc.declare_dram_parameter("W23", [P, D], bf16, isOutput=False)
    M1R_d = nc.declare_dram_parameter("M1R", [GPC, 1, N], bf16, isOutput=False)
    SC_d = nc.declare_dram_parameter("SC", [GPC, P, NB], f32, isOutput=False)
    IOB_d = nc.declare_dram_parameter("IOB", [P, P], bf16, isOutput=False)
    out_d = nc.declare_dram_parameter("out", [GPC, D, N], bf16, isOutput=True)

    with tile.TileContext(nc) as tc, ExitStack() as ctx:
        wpool = ctx.enter_context(tc.tile_pool(name="wpool", bufs=1))
        ahp = ctx.enter_context(tc.tile_pool(name="ahp", bufs=1))
        atp = ctx.enter_context(tc.tile_pool(name="atp", bufs=1))
        cpool = ctx.enter_context(tc.tile_pool(name="cpool", bufs=2))
        wqp = ctx.enter_context(tc.tile_pool(name="wqp", bufs=1))
        epool = ctx.enter_context(tc.tile_pool(name="epool", bufs=1))
        mpool = ctx.enter_context(tc.tile_pool(name="mpool", bufs=1))
        ogp = ctx.enter_context(tc.tile_pool(name="ogp", bufs=2))
        ps = ctx.enter_context(tc.tile_pool(name="ps", bufs=1, space="PSUM"))

        # ---- small resident tensors ----
        W23 = wpool.tile([P, D], bf16)
        nc.sync.dma_start(W23[:], W23_d.ap())
        iob = wpool.tile([P, P], bf16)
        nc.sync.dma_start(iob[:], IOB_d.ap())
        wg, sc, m1r = {}, {}, {}
        for g in range(GPC):
            wg[g] = wpool.tile([P, KB, 2, D], fp8, tag=f"wg{g}", name=f"wg{g}")
            nc.sync.dma_start(wg[g][:],
                              W1G_d.ap()[g].rearrange("kb p i d -> p kb i d"))
            sc[g] = wpool.tile([P, NB], f32, tag=f"sc{g}", name=f"sc{g}")
            nc.sync.dma_start(sc[g][:], SC_d.ap()[g])
            m1r[g] = wpool.tile([1, N], bf16, tag=f"m1{g}", name=f"m1{g}")
            nc.sync.dma_start(m1r[g][:], M1R_d.ap()[g])
        ones8 = wpool.tile([P, 2, 1], fp8)
        nc.vector.memset(ones8[:], 1.0)
        jW = wpool.tile([P, 2, P], fp8, tag="jW", name="jW")
        nc.vector.memset(jW[:], 0.0)
        jR = wpool.tile([P, 2, CH], fp8, tag="jR", name="jR")
        nc.vector.memset(jR[:], 0.0)

        # ---- input streams: strips then natural, g0 then g1 ----
        AT, AH = {}, {}
        for g in range(GPC):
            AT[g] = [atp.tile([P, KB, 2, CH], fp8, tag=f"at{g}{r}",
                              name=f"at{g}{r}") for r in range(NCH)]
            AH[g] = [ahp.tile([P, KB, 2, CH], fp8, tag=f"ah{g}{c}",
                              name=f"ah{g}{c}") for c in range(NCH)]
        for g in range(GPC):
            for r in range(NCH):
                nc.sync.dma_start(AT[g][r][:], AT_d.ap()[g][r])
            for c in range(NCH):
                nc.sync.dma_start(AH[g][c][:], AH_d.ap()[g][c])

        out_ap = out_d.ap()

        ptag = {0: [f"a{i}" for i in range(NCH)], 1: [f"b{i}" for i in range(NCH)]}
        fillt = {}

        def fill(n, g_idle):
            """n junk DR matmuls (~107ns each at full clock) to keep the PE
            queue gapless; writes a scratch bank of the idle graph."""
            t = fillt.get(g_idle)
            if t is None:
                t = ps.tile([P, CH], f32, tag=ptag[g_idle][3],
                            name=f"fill{g_idle}")
                fillt[g_idle] = t
            for _ in range(n):
                nc.tensor.matmul(t[:], jW[:], jR[:], start=True, stop=True,
                                 perf_mode=DR, skip_group_check=True)

        w8 = {}       # current fp8 lhsT per graph
        eps = {}      # quantization residual, node-major bf16
        murow = {}    # mu row [1, D] bf16 per (g, step)

        def u_mms(g, rch):
            ups = ps.tile([P, CH], f32, tag=ptag[g][rch], name=f"ups{g}{rch}")
            for cb in range(KB):
                nc.tensor.matmul(ups[:], wg[g][:, cb], AT[g][rch][:, cb],
                                 start=(cb == 0), stop=(cb == KB - 1),
                                 perf_mode=DR)
            return ups

        def _mk_tq(g, ch, ups, name):
            """Transpose the feature-major chunk into node-major quads.
            Chunks 0-2 go through the XBAR DMA transpose (frees the PE);
            the boundary-critical last chunk uses PE transposes (lower
            latency into the next phase)."""
            if ch < NCH - 1:
                tf = cpool.tile([P, CH], bf16, tag=f"tf{g}{ch % 2}",
                                name=f"tf{name}")
                nc.vector.tensor_copy(tf[:], ups[:])
                tq = cpool.tile([P, 4, P], bf16, tag=f"tq{g}{ch % 2}",
                                name=f"tq{name}")
                nc.scalar.dma_start(tq[:], tf[:], transpose=True)
            else:
                tf = cpool.tile([P, CH], bf16, tag=f"tf{g}{ch % 2}",
                                name=f"tf{name}")
                nc.vector.tensor_copy(tf[:], ups[:])
                tq = ps.tile([P, 4, P], bf16, tag=ptag[g][ch], name=f"tqp{name}")
                for j in range(4):
                    nc.tensor.transpose(tq[:, j], tf[:, j * P:(j + 1) * P],
                                        iob[:])
            return tq

        def u_post(g, rch, ups, w):
            tq = _mk_tq(g, rch, ups, f"u{g}{rch}")
            for j in range(4):
                nb = 4 * rch + j
                nc.scalar.activation(w[:, nb], tq[:, j], COPY,
                                     scale=sc[g][:, nb, None])

        def t_mms(g, step, ch):
            tps = ps.tile([P, CH], f32, tag=ptag[g][ch], name=f"t{step}{g}{ch}")
            for kb in range(KB):
                nc.tensor.matmul(tps[:], w8[g][:, 2 * kb:2 * kb + 2],
                                 AH[g][ch][:, kb],
                                 start=(kb == 0), stop=(kb == KB - 1),
                                 perf_mode=DR)
                if step > 1 and kb == KB - 2:
                    # rank-1 mu correction, accumulated inside the group
                    nc.tensor.matmul(tps[:], murow[(g, step - 1)],
                                     m1r[g][:, ch * CH:(ch + 1) * CH],
                                     start=False, stop=False,
                                     skip_group_check=True)
            return tps

        def t_post(g, step, ch, tps, wn, ep):
            sl = slice(ch * 4, ch * 4 + 4)
            if step == 3:
                # feature-major epilogue: out^T-chunk = W23^T @ tc3 in one
                # 512-col matmul; the host transposes the final output.
                tc3 = cpool.tile([P, CH], bf16, tag=f"c3{g}", name=f"tc3{g}{ch}")
                nc.scalar.activation(tc3[:], tps[:], COPY, scale=2.0 ** -9)
                epo = ps.tile([P, CH], f32, tag=ptag[g][ch], name=f"epo{g}{ch}")
                nc.tensor.matmul(epo[:], W23[:], tc3[:], start=True, stop=True)
                og = ogp.tile([P, CH], bf16, tag=f"og{g}", name=f"og{g}{ch}")
                nc.vector.tensor_scalar_mul(og[:], epo[:], 2.0 ** -15)
                nc.sync.dma_start(out_ap[g][:, ch * CH:(ch + 1) * CH], og[:])
                return
            tq = _mk_tq(g, ch, tps, f"t{g}{step}{ch}")
            nc.scalar.activation(wn[:, sl], tq[:], COPY, scale=2.0 ** -16)
            nc.vector.scalar_tensor_tensor(ep[:, sl], tq[:], 2.0 ** -16,
                                           wn[:, sl], MUL, SUB)

        def phase_u(g, g_idle, nfill):
            w = wqp.tile([P, NB, D], fp8, tag=f"w{g}0", name=f"w0{g}")
            pend = None
            for rch in range(NCH):
                ups = u_mms(g, rch)
                if pend is not None:
                    u_post(g, pend[0], pend[1], w)
                pend = (rch, ups)
                if rch < NCH - 1:
                    fill(nfill, g_idle)
            u_post(g, pend[0], pend[1], w)
            w8[g] = w

        def phase_t(g, g_idle, step, nfill):
            wn = en = None
            if step < 3:
                wn = wqp.tile([P, NB, D], fp8, tag=f"w{g}{step % 2}",
                              name=f"w{step}{g}")
                en = epool.tile([P, NB, D], fp8, tag=f"eps{g}",
                                name=f"eps{step}{g}")
            pend = None
            for ch in range(NCH):
                tps = t_mms(g, step, ch)
                if pend is not None:
                    t_post(g, step, pend[0], pend[1], wn, en)
                pend = (ch, tps)
                if nfill and ch < NCH - 1:
                    fill(nfill, g_idle)
            t_post(g, step, pend[0], pend[1], wn, en)
            if step < 3:
                w8[g], eps[g] = wn, en

        def phase_mu(g, step):
            """mu = 2^-11 * colsum(eps) as a [1, D] bf16 row (fp8 DR mms)."""
            muT = ps.tile([P, 1], f32, tag=ptag[g][0], name=f"muT{g}{step}")
            for m in range(KB):
                nc.tensor.matmul(muT[:], eps[g][:, 2 * m:2 * m + 2], ones8[:],
                                 start=(m == 0), stop=(m == KB - 1),
                                 perf_mode=DR)
            muTs = mpool.tile([P, 1], bf16, tag=f"muTs{g}", name=f"muTs{g}{step}")
            nc.vector.tensor_scalar_mul(muTs[:], muT[:], 2.0 ** -11)
            rowp = ps.tile([1, P], bf16, tag=ptag[g][0], name=f"murp{g}{step}")
            nc.tensor.transpose(rowp[:], muTs[:], iob[:])
            row = mpool.tile([1, P], bf16, tag=f"mur{g}{step}",
                             name=f"mur{g}{step}")
            nc.vector.tensor_copy(row[:], rowp[:])
            murow[(g, step)] = row

        # ---- schedule ----
        fill(FILL["head"], 1)
        phase_u(0, 1, FILL["u0"])
        fill(FILL["u0t1"], 1)
        phase_t(0, 1, 1, FILL["t1"])
        fill(FILL["t1mu"], 1)
        phase_mu(0, 1)
        phase_t(0, 1, 2, 0)
        phase_mu(0, 2)
        if FILL["t2t3"]:
            fill(FILL["t2t3"], 1)
        # U(1) chunks interleave with T3(0): strips for c0/c1 land around
        # when w2(0) posts finish, and T3(0) fills the strip-arrival stalls.
        w1t = wqp.tile([P, NB, D], fp8, tag="w10", name="w01")
        u0 = u_mms(1, 0)
        u1 = u_mms(1, 1)
        u_post(1, 0, u0, w1t)
        phase_t(0, 1, 3, 0)
        u_post(1, 1, u1, w1t)
        u2 = u_mms(1, 2)
        u_post(1, 2, u2, w1t)
        fill(FILL["u1b"], 0)
        u3 = u_mms(1, 3)
        u_post(1, 3, u3, w1t)
        w8[1] = w1t
        fill(FILL["u1t1"], 0)
        phase_t(1, 0, 1, FILL["t1"])
        fill(FILL["t1mu"], 0)
        phase_mu(1, 1)
        phase_t(1, 0, 2, 0)
        phase_mu(1, 2)
        fill(FILL["t3tail"], 0)
        phase_t(1, 0, 3, 0)

    nc.compile()
    return nc


def _get_nc():
    if "nc" not in _COMPILED:
        _COMPILED["nc"] = _build()
    return _COMPILED["nc"]


FP8 = ml_dtypes.float8_e4m3
BF16 = ml_dtypes.bfloat16


def _q8(x):
    return np.clip(x, -240.0, 240.0).astype(FP8)


def _dither_q8(xs):
    """Per-column fp8 quantization with near-zero column error means."""
    q = _q8(xs)
    qf = q.astype(np.float32)
    r = xs - qf
    m = r.sum(0)
    s = np.where(m >= 0, 1.0, -1.0).astype(np.float32)
    u = q.view(np.uint8)
    mag = (u & 0x7F).astype(np.int16)
    neg = (u & 0x80) != 0
    dirpos = np.broadcast_to(s > 0, xs.shape)
    away = (~neg) == dirpos
    nmag = np.where(mag == 0, 1, np.where(away, mag + 1, mag - 1))
    nsign = np.where(mag == 0, ~dirpos, neg)
    nb = ((nmag.astype(np.uint8) & 0x7F) | (nsign.astype(np.uint8) << 7))
    nxt = nb.view(FP8).astype(np.float32)
    ok = np.isfinite(nxt) & (np.abs(nxt) <= 240.0) & (nmag <= 0x7E)
    step = np.where(ok, nxt - qf, 0.0)
    key = np.where(ok, r * s[None, :], -np.inf)
    order = np.argsort(-key, axis=0)
    step_sorted = np.take_along_axis(step, order, axis=0)
    cum = np.cumsum(step_sorted, axis=0)
    err = np.abs(m[None, :] - cum)
    k = np.argmin(np.vstack([np.abs(m)[None, :], err]), axis=0)
    out = qf.copy()
    for d in range(xs.shape[1]):
        if k[d] > 0:
            idx = order[:k[d], d]
            out[idx, d] = nxt[idx, d]
    return out.astype(FP8)


def _pack(a):
    """[B, N(rows), N(cols)] -> [B, NCH, P, KB, 2, CH]: rows r = 256kb+128i+p
    packed DoubleRow, cols c = 512ch + cc chunked."""
    x = a.reshape(B, KB, 2, P, NCH, CH)
    return np.ascontiguousarray(x.transpose(0, 4, 3, 1, 2, 5))


def kernel(flows, W1, b1, W2, b2, W3, b3, _trace=False):
    from concourse.bass_utils import run_bass_kernel_spmd

    flows = np.asarray(flows, dtype=np.float32)
    W1 = np.asarray(W1, dtype=np.float32)
    W2 = np.asarray(W2, dtype=np.float32)
    W3 = np.asarray(W3, dtype=np.float32)
    b1 = np.asarray(b1, dtype=np.float32)
    b2 = np.asarray(b2, dtype=np.float32)
    b3 = np.asarray(b3, dtype=np.float32)

    nc = _get_nc()

    deg = flows.sum(axis=1)                          # [B, N] column sums
    dinv = (1.0 / np.sqrt(deg)).astype(np.float32)

    # Ahat = D A D, quantized at 2^16; natural + transposed packings
    Aq8 = np.empty((B, N, N), dtype=FP8)
    M1R = np.empty((B, 1, N), dtype=BF16)
    for g in range(B):
        ah = (flows[g] * (dinv[g][:, None] * 2.0 ** 16)) * dinv[g][None, :]
        Aq8[g] = _q8(ah)
        M1R[g, 0] = Aq8[g].astype(np.float32).sum(axis=0).astype(BF16)
    AHp = _pack(Aq8)
    ATp = _pack(np.ascontiguousarray(Aq8.transpose(0, 2, 1)))

    # W1g = 2^6 D^{-1} W1 per graph, dither-quantized per column
    W1g = (np.sqrt(deg)[:, :, None] * W1[None, :, :] * 2.0 ** 6).astype(np.float32)
    W1q = _dither_q8(W1g.transpose(1, 0, 2).reshape(N, B * D))
    W1q = W1q.reshape(N, B, D).transpose(1, 0, 2)    # [B, N, D] fp8
    W1G = np.ascontiguousarray(
        W1q.reshape(B, KB, 2, P, D).transpose(0, 1, 3, 2, 4))

    W23 = ((W2 @ W3) * 2.0 ** 2).astype(BF16)
    SC = (np.sqrt(deg) * 2.0 ** -16).astype(np.float32)       # [B, N]
    SC = np.ascontiguousarray(SC.reshape(B, NB, P).transpose(0, 2, 1))

    in_maps = []
    for c in range(NCORES):
        sl = slice(c * GPC, (c + 1) * GPC)
        in_maps.append({
            "AH": AHp[sl], "AT": ATp[sl],
            "W1G": W1G[sl], "W23": W23,
            "M1R": M1R[sl], "SC": SC[sl],
            "IOB": np.eye(P, dtype=BF16),
        })

    res = run_bass_kernel_spmd(nc, in_maps, core_ids=list(range(NCORES)),
                               trace=_trace)
    out = np.concatenate([res.results[c]["out"] for c in range(NCORES)], axis=0)
    out = np.ascontiguousarray(out.astype(np.float32).transpose(0, 2, 1))

    if np.any(b1) or np.any(b2) or np.any(b3):
        dv = np.where(deg > 0, 1.0 / np.sqrt(deg), 0.0).astype(np.float32)
        m1 = dv * np.einsum('brc,br->bc', flows, dv)
        m2 = dv * np.einsum('brc,br->bc', flows, dv * m1)
        out += m2[..., None] * (b1 @ W2 @ W3)[None, None, :]
        out += m1[..., None] * (b2 @ W3)[None, None, :]
        out += b3[None, None, :]

    if _trace:
        return out, res
    return out


# revision 17
# speedup vs baseline: 1.5417x; 1.5417x over previous
"""Trainium2 Bass kernel for nn_Encoder_Flows (3-layer dense GCN message passing).

Math per graph (reference):
    A = flows [N, N];  deg[c] = sum_r A[r, c];  dinv = rsqrt(deg)
    L(x, W, b) = dinv * (A^T @ (dinv * (x @ W))) + b
    out = L(L(L(A, W1, b1), W2, b2), W3, b3)          # [N, 128]

Algebra: with M = diag(dinv) A^T diag(dinv), node-dim M commutes with the
feature-dim weights, so (bias-free) out = M^3 (A W1) (W2 W3).  Key trick of
this version: the degree normalization is folded into the shipped matrix on
the host:  Ahat = D A D  =>  M = Ahat^T exactly.  Every M-apply is then a
plain fp8 DoubleRow matmul chain  t_k = Ahat^T w_{k-1}  with NO per-step
dinv scaling on device (the vector engine only does psum->sbuf casts and the
eps residuals).  The U phase u = A W1 runs off Ahat^T strips with
W1g = D^{-1} W1 folded on the host: (D^{-1}W1)^T Ahat^T = u^T D, undone by a
per-node act scale when u is quantized.

fp8 quantization corrections (node-mean noise is amplified ~sqrt(N) by the
adjacency's Perron mode):
  - W1g is dither-quantized per column (error col-sums ~0).
  - w1/w2's quantization residual col-means mu_k are measured on device and
    applied as exact rank-1 psum accumulations  m1 (x) mu_k  into the next
    phase, where m1 = colsum(Ahat_q) is shipped from the host (a K=1 matmul
    appended to the accumulation group -- no vector work).
Scales: Ahat*2^16, W1g*2^6, w*2^7(t units), mu*2^-11, W23*2^2, out=psum*2^-15.

Performance design (measured baseline: PE never left the 1.2GHz mid p-state;
the 2.4GHz p-state needs >3us of gapless tensor-queue execution):
  - data: 2 graphs/core, Ahat shipped in both layouts (natural row-packed +
    transposed strips), 1MB c-chunk DMAs so each phase is chunk-paced;
    output in bf16.  ~17.9MB on a ~400GB/s DMA bus dominates the schedule.
  - the tensor queue is padded with junk DoubleRow "filler" matmuls wherever
    it would otherwise idle (DMA-paced stretches, post-processing trails) so
    the PE holds the full 2.4GHz clock; at full clock a 512-col DR matmul
    retires in ~107ns.
  - posts per chunk: vector casts psum->bf16, PE transposes 128x128 blocks,
    scalar quantizes to fp8 (act, with the per-node scale vector for U),
    vector computes eps = x - w in one scalar_tensor_tensor op.
"""

import sys
from contextlib import ExitStack

import numpy as np

for _p in ("/opt/trn_rl_repo", "/opt/pypackages"):
    if _p not in sys.path:
        sys.path.append(_p)

import ml_dtypes

B, N, P = 16, 2048, 128
NB = N // P          # 16 node 128-blocks
KB = N // (2 * P)    # 8 DoubleRow pair-blocks (256 rows each)
NCORES = 8
GPC = B // NCORES    # graphs per core
D = 128              # feature width carried through the fused chain
CH = 512             # psum chunk (one bank of fp32)
NCH = N // CH

_COMPILED = {}

# filler counts (each ~107ns of junk DR matmul at full clock), tuned to the
# DMA pacing of the schedule
FILL = dict(head=25, u0=4, u0t1=5, t1=4, t1mu=3, t2t3=0,
            u1a=0, u1b=4, u1t1=3, t3tail=3)


def _build():
    import concourse.mybir as mybir
    import concourse.tile as tile
    from concourse import bacc

    f32 = mybir.dt.float32
    bf16 = mybir.dt.bfloat16
    fp8 = mybir.dt.float8e4
    DR = mybir.MatmulPerfMode.DoubleRow
    MUL = mybir.AluOpType.mult
    SUB = mybir.AluOpType.subtract
    COPY = mybir.ActivationFunctionType.Copy

    nc = bacc.Bacc("TRN2", target_bir_lowering=False)
    AH_d = nc.declare_dram_parameter("AH", [GPC, NCH, P, KB, 2, CH], fp8,
                                     isOutput=False)
    AT_d = nc.declare_dram_parameter("AT", [GPC, NCH, P, KB, 2, CH], fp8,
                                     isOutput=False)
    W1G_d = nc.declare_dram_parameter("W1G", [GPC, KB, P, 2, D], fp8,
                                      isOutput=False)
    W23_d = nc.declare_dram_parameter("W23", [P, D], bf16, isOutput=False)
    M1R_d = nc.declare_dram_parameter("M1R", [GPC, 1, N], bf16, isOutput=False)
    SC_d = nc.declare_dram_parameter("SC", [GPC, P, NB], f32, isOutput=False)
    IOB_d = nc.declare_dram_parameter("IOB", [P, P], bf16, isOutput=False)
    out_d = nc.declare_dram_parameter("out", [GPC, D, N], bf16, isOutput=True)

    with tile.TileContext(nc) as tc, ExitStack() as ctx:
        wpool = ctx.enter_context(tc.tile_pool(name="wpool", bufs=1))
        ahp = ctx.enter_context(tc.tile_pool(name="ahp", bufs=1))
        atp = ctx.enter_context(tc.tile_pool(name="atp", bufs=1))
        cpool = ctx.enter_context(tc.tile_pool(name="cpool", bufs=2))
        wqp = ctx.enter_context(tc.tile_pool(name="wqp", bufs=1))
        epool = ctx.enter_context(tc.tile_pool(name="epool", bufs=1))
        mpool = ctx.enter_context(tc.tile_pool(name="mpool", bufs=1))
        ogp = ctx.enter_context(tc.tile_pool(name="ogp", bufs=2))
        ps = ctx.enter_context(tc.tile_pool(name="ps", bufs=1, space="PSUM"))

        # ---- small resident tensors ----
        W23 = wpool.tile([P, D], bf16)
        nc.sync.dma_start(W23[:], W23_d.ap())
        iob = wpool.tile([P, P], bf16)
        nc.sync.dma_start(iob[:], IOB_d.ap())
        wg, sc, m1r = {}, {}, {}
        for g in range(GPC):
            wg[g] = wpool.tile([P, KB, 2, D], fp8, tag=f"wg{g}", name=f"wg{g}")
            nc.sync.dma_start(wg[g][:],
                              W1G_d.ap()[g].rearrange("kb p i d -> p kb i d"))
            sc[g] = wpool.tile([P, NB], f32, tag=f"sc{g}", name=f"sc{g}")
            nc.sync.dma_start(sc[g][:], SC_d.ap()[g])
            m1r[g] = wpool.tile([1, N], bf16, tag=f"m1{g}", name=f"m1{g}")
            nc.sync.dma_start(m1r[g][:], M1R_d.ap()[g])
        ones8 = wpool.tile([P, 2, 1], fp8)
        nc.vector.memset(ones8[:], 1.0)
        jW = wpool.tile([P, 2, P], fp8, tag="jW", name="jW")
        nc.vector.memset(jW[:], 0.0)
        jR = wpool.tile([P, 2, CH], fp8, tag="jR", name="jR")
        nc.vector.memset(jR[:], 0.0)

        # ---- input streams: strips then natural, g0 then g1 ----
        AT, AH = {}, {}
        for g in range(GPC):
            AT[g] = [atp.tile([P, KB, 2, CH], fp8, tag=f"at{g}{r}",
                              name=f"at{g}{r}") for r in range(NCH)]
            AH[g] = [ahp.tile([P, KB, 2, CH], fp8, tag=f"ah{g}{c}",
                              name=f"ah{g}{c}") for c in range(NCH)]
        for g in range(GPC):
            for r in range(NCH):
                nc.sync.dma_start(AT[g][r][:], AT_d.ap()[g][r])
            for c in range(NCH):
                nc.sync.dma_start(AH[g][c][:], AH_d.ap()[g][c])

        out_ap = out_d.ap()

        ptag = {0: [f"a{i}" for i in range(NCH)], 1: [f"b{i}" for i in range(NCH)]}
        fillt = {}

        def fill(n, g_idle):
            """n junk DR matmuls (~107ns each at full clock) to keep the PE
            queue gapless; writes a scratch bank of the idle graph."""
            t = fillt.get(g_idle)
            if t is None:
                t = ps.tile([P, CH], f32, tag=ptag[g_idle][3],
                            name=f"fill{g_idle}")
                fillt[g_idle] = t
            for _ in range(n):
                nc.tensor.matmul(t[:], jW[:], jR[:], start=True, stop=True,
                                 perf_mode=DR, skip_group_check=True)

        w8 = {}       # current fp8 lhsT per graph
        eps = {}      # quantization residual, node-major bf16
        murow = {}    # mu row [1, D] bf16 per (g, step)

        def u_mms(g, rch):
            ups = ps.tile([P, CH], f32, tag=ptag[g][rch], name=f"ups{g}{rch}")
            for cb in range(KB):
                nc.tensor.matmul(ups[:], wg[g][:, cb], AT[g][rch][:, cb],
                                 start=(cb == 0), stop=(cb == KB - 1),
                                 perf_mode=DR)
            return ups

        def _mk_tq(g, ch, ups, name):
            """Transpose the feature-major chunk into node-major quads on
            the PE (XBAR DMA transpose measured ~1.4us issue cost on the
            scalar queue -- far worse)."""
            tf = cpool.tile([P, CH], bf16, tag=f"tf{g}{ch % 2}",
                            name=f"tf{name}")
            nc.vector.tensor_copy(tf[:], ups[:])
            tq = ps.tile([P, 4, P], bf16, tag=ptag[g][ch], name=f"tqp{name}")
            for j in range(4):
                nc.tensor.transpose(tq[:, j], tf[:, j * P:(j + 1) * P],
                                    iob[:])
            return tq

        def u_post(g, rch, ups, w):
            tq = _mk_tq(g, rch, ups, f"u{g}{rch}")
            for j in range(4):
                nb = 4 * rch + j
                nc.scalar.activation(w[:, nb], tq[:, j], COPY,
                                     scale=sc[g][:, nb, None])

        def t_mms(g, step, ch):
            tps = ps.tile([P, CH], f32, tag=ptag[g][ch], name=f"t{step}{g}{ch}")
            for kb in range(KB):
                nc.tensor.matmul(tps[:], w8[g][:, 2 * kb:2 * kb + 2],
                                 AH[g][ch][:, kb],
                                 start=(kb == 0), stop=(kb == KB - 1),
                                 perf_mode=DR)
                if step > 1 and kb == KB - 2:
                    # rank-1 mu correction, accumulated inside the group
                    nc.tensor.matmul(tps[:], murow[(g, step - 1)],
                                     m1r[g][:, ch * CH:(ch + 1) * CH],
                                     start=False, stop=False,
                                     skip_group_check=True)
            return tps

        def t_post(g, step, ch, tps, wn, ep):
            sl = slice(ch * 4, ch * 4 + 4)
            if step == 3:
                # feature-major epilogue: out^T-chunk = W23^T @ tc3 in one
                # 512-col matmul; the host transposes the final output.
                tc3 = cpool.tile([P, CH], bf16, tag=f"c3{g}", name=f"tc3{g}{ch}")
                nc.scalar.activation(tc3[:], tps[:], COPY, scale=2.0 ** -9)
                epo = ps.tile([P, CH], f32, tag=ptag[g][ch], name=f"epo{g}{ch}")
                nc.tensor.matmul(epo[:], W23[:], tc3[:], start=True, stop=True)
                og = ogp.tile([P, CH], bf16, tag=f"og{g}", name=f"og{g}{ch}")
                nc.vector.tensor_scalar_mul(og[:], epo[:], 2.0 ** -15)
                nc.sync.dma_start(out_ap[g][:, ch * CH:(ch + 1) * CH], og[:])
                return
            tq = _mk_tq(g, ch, tps, f"t{g}{step}{ch}")
            nc.scalar.activation(wn[:, sl], tq[:], COPY, scale=2.0 ** -16)
            nc.vector.scalar_tensor_tensor(ep[:, sl], tq[:], 2.0 ** -16,
                                           wn[:, sl], MUL, SUB)

        def phase_u(g, g_idle, nfill):
            w = wqp.tile([P, NB, D], fp8, tag=f"w{g}0", name=f"w0{g}")
            pend = None
            for rch in range(NCH):
                ups = u_mms(g, rch)
                if pend is not None:
                    u_post(g, pend[0], pend[1], w)
                pend = (rch, ups)
                if rch < NCH - 1:
                    fill(nfill, g_idle)
            u_post(g, pend[0], pend[1], w)
            w8[g] = w

        def phase_t(g, g_idle, step, nfill):
            wn = en = None
            if step < 3:
                wn = wqp.tile([P, NB, D], fp8, tag=f"w{g}{step % 2}",
                              name=f"w{step}{g}")
                en = epool.tile([P, NB, D], fp8, tag=f"eps{g}",
                                name=f"eps{step}{g}")
            pend = None
            for ch in range(NCH):
                tps = t_mms(g, step, ch)
                if pend is not None:
                    t_post(g, step, pend[0], pend[1], wn, en)
                pend = (ch, tps)
                if nfill and ch < NCH - 1:
                    fill(nfill, g_idle)
            t_post(g, step, pend[0], pend[1], wn, en)
            if step < 3:
                w8[g], eps[g] = wn, en

        def phase_mu(g, step):
            """mu = 2^-11 * colsum(eps) as a [1, D] bf16 row (fp8 DR mms)."""
            muT = ps.tile([P, 1], f32, tag=ptag[g][0], name=f"muT{g}{step}")
            for m in range(KB):
                nc.tensor.matmul(muT[:], eps[g][:, 2 * m:2 * m + 2], ones8[:],
                                 start=(m == 0), stop=(m == KB - 1),
                                 perf_mode=DR)
            muTs = mpool.tile([P, 1], bf16, tag=f"muTs{g}", name=f"muTs{g}{step}")
            nc.vector.tensor_scalar_mul(muTs[:], muT[:], 2.0 ** -11)
            rowp = ps.tile([1, P], bf16, tag=ptag[g][0], name=f"murp{g}{step}")
            nc.tensor.transpose(rowp[:], muTs[:], iob[:])
            row = mpool.tile([1, P], bf16, tag=f"mur{g}{step}",
                             name=f"mur{g}{step}")
            nc.vector.tensor_copy(row[:], rowp[:])
            murow[(g, step)] = row

        # ---- schedule ----
        fill(FILL["head"], 1)
        phase_u(0, 1, FILL["u0"])
        fill(FILL["u0t1"], 1)
        phase_t(0, 1, 1, FILL["t1"])
        fill(FILL["t1mu"], 1)
        phase_mu(0, 1)
        phase_t(0, 1, 2, 0)
        phase_mu(0, 2)
        if FILL["t2t3"]:
            fill(FILL["t2t3"], 1)
        # U(1) chunks interleave with T3(0): strips for c0/c1 land around
        # when w2(0) posts finish, and T3(0) fills the strip-arrival stalls.
        w1t = wqp.tile([P, NB, D], fp8, tag="w10", name="w01")
        u0 = u_mms(1, 0)
        u1 = u_mms(1, 1)
        u_post(1, 0, u0, w1t)
        phase_t(0, 1, 3, 0)
        u_post(1, 1, u1, w1t)
        u2 = u_mms(1, 2)
        u_post(1, 2, u2, w1t)
        fill(FILL["u1b"], 0)
        u3 = u_mms(1, 3)
        u_post(1, 3, u3, w1t)
        w8[1] = w1t
        fill(FILL["u1t1"], 0)
        phase_t(1, 0, 1, FILL["t1"])
        fill(FILL["t1mu"], 0)
        phase_mu(1, 1)
        phase_t(1, 0, 2, 0)
        phase_mu(1, 2)
        fill(FILL["t3tail"], 0)
        phase_t(1, 0, 3, 0)

    nc.compile()
    return nc


def _get_nc():
    if "nc" not in _COMPILED:
        _COMPILED["nc"] = _build()
    return _COMPILED["nc"]


FP8 = ml_dtypes.float8_e4m3
BF16 = ml_dtypes.bfloat16


def _q8(x):
    return np.clip(x, -240.0, 240.0).astype(FP8)


def _dither_q8(xs):
    """Per-column fp8 quantization with near-zero column error means."""
    q = _q8(xs)
    qf = q.astype(np.float32)
    r = xs - qf
    m = r.sum(0)
    s = np.where(m >= 0, 1.0, -1.0).astype(np.float32)
    u = q.view(np.uint8)
    mag = (u & 0x7F).astype(np.int16)
    neg = (u & 0x80) != 0
    dirpos = np.broadcast_to(s > 0, xs.shape)
    away = (~neg) == dirpos
    nmag = np.where(mag == 0, 1, np.where(away, mag + 1, mag - 1))
    nsign = np.where(mag == 0, ~dirpos, neg)
    nb = ((nmag.astype(np.uint8) & 0x7F) | (nsign.astype(np.uint8) << 7))
    nxt = nb.view(FP8).astype(np.float32)
    ok = np.isfinite(nxt) & (np.abs(nxt) <= 240.0) & (nmag <= 0x7E)
    step = np.where(ok, nxt - qf, 0.0)
    key = np.where(ok, r * s[None, :], -np.inf)
    order = np.argsort(-key, axis=0)
    step_sorted = np.take_along_axis(step, order, axis=0)
    cum = np.cumsum(step_sorted, axis=0)
    err = np.abs(m[None, :] - cum)
    k = np.argmin(np.vstack([np.abs(m)[None, :], err]), axis=0)
    out = qf.copy()
    for d in range(xs.shape[1]):
        if k[d] > 0:
            idx = order[:k[d], d]
            out[idx, d] = nxt[idx, d]
    return out.astype(FP8)


def _pack(a):
    """[B, N(rows), N(cols)] -> [B, NCH, P, KB, 2, CH]: rows r = 256kb+128i+p
    packed DoubleRow, cols c = 512ch + cc chunked."""
    x = a.reshape(B, KB, 2, P, NCH, CH)
    return np.ascontiguousarray(x.transpose(0, 4, 3, 1, 2, 5))


def kernel(flows, W1, b1, W2, b2, W3, b3, _trace=False):
    from concourse.bass_utils import run_bass_kernel_spmd

    flows = np.asarray(flows, dtype=np.float32)
    W1 = np.asarray(W1, dtype=np.float32)
    W2 = np.asarray(W2, dtype=np.float32)
    W3 = np.asarray(W3, dtype=np.float32)
    b1 = np.asarray(b1, dtype=np.float32)
    b2 = np.asarray(b2, dtype=np.float32)
    b3 = np.asarray(b3, dtype=np.float32)

    nc = _get_nc()

    deg = flows.sum(axis=1)                          # [B, N] column sums
    dinv = (1.0 / np.sqrt(deg)).astype(np.float32)

    # Ahat = D A D, quantized at 2^16; natural + transposed packings
    Aq8 = np.empty((B, N, N), dtype=FP8)
    M1R = np.empty((B, 1, N), dtype=BF16)
    for g in range(B):
        ah = (flows[g] * (dinv[g][:, None] * 2.0 ** 16)) * dinv[g][None, :]
        Aq8[g] = _q8(ah)
        M1R[g, 0] = Aq8[g].astype(np.float32).sum(axis=0).astype(BF16)
    AHp = _pack(Aq8)
    ATp = _pack(np.ascontiguousarray(Aq8.transpose(0, 2, 1)))

    # W1g = 2^6 D^{-1} W1 per graph, dither-quantized per column
    W1g = (np.sqrt(deg)[:, :, None] * W1[None, :, :] * 2.0 ** 6).astype(np.float32)
    W1q = _dither_q8(W1g.transpose(1, 0, 2).reshape(N, B * D))
    W1q = W1q.reshape(N, B, D).transpose(1, 0, 2)    # [B, N, D] fp8
    W1G = np.ascontiguousarray(
        W1q.reshape(B, KB, 2, P, D).transpose(0, 1, 3, 2, 4))

    W23 = ((W2 @ W3) * 2.0 ** 2).astype(BF16)
    SC = (np.sqrt(deg) * 2.0 ** -16).astype(np.float32)       # [B, N]
    SC = np.ascontiguousarray(SC.reshape(B, NB, P).transpose(0, 2, 1))

    in_maps = []
    for c in range(NCORES):
        sl = slice(c * GPC, (c + 1) * GPC)
        in_maps.append({
            "AH": AHp[sl], "AT": ATp[sl],
            "W1G": W1G[sl], "W23": W23,
            "M1R": M1R[sl], "SC": SC[sl],
            "IOB": np.eye(P, dtype=BF16),
        })

    res = run_bass_kernel_spmd(nc, in_maps, core_ids=list(range(NCORES)),
                               trace=_trace)
    out = np.concatenate([res.results[c]["out"] for c in range(NCORES)], axis=0)
    out = np.ascontiguousarray(out.astype(np.float32).transpose(0, 2, 1))

    if np.any(b1) or np.any(b2) or np.any(b3):
        dv = np.where(deg > 0, 1.0 / np.sqrt(deg), 0.0).astype(np.float32)
        m1 = dv * np.einsum('brc,br->bc', flows, dv)
        m2 = dv * np.einsum('brc,br->bc', flows, dv * m1)
        out += m2[..., None] * (b1 @ W2 @ W3)[None, None, :]
        out += m1[..., None] * (b2 @ W3)[None, None, :]
        out += b3[None, None, :]

    if _trace:
        return out, res
    return out
